# revision 3
# baseline (speedup 1.0000x reference)
"""Trainium2 Bass kernel for nn_ClustGeoNodeEncoder (segment_reduce).

Strategy (data-parallel over the cluster axis, per the sharding hint):
  - Host packs the voxel features as 8-f32 rows: x, y, z, value,
    onehot(sem==1..4); count of sem==0 is recovered as n - sum(oh1..4).
  - Clusters are sorted by length and dealt round-robin to the 8 cores so
    every core compiles the same program (SPMD): 32 tiles x 128 clusters
    per core, tile t padded to Lb[t] = max length in its global rank range.
  - The host materializes the per-core dense element stream
    gst[P, S*8] (cluster p's elements for tile t at columns
    off_t*8:(off_t+lb)*8, padded slots all-zero).  The device streams it
    with plain contiguous DMAs -- no per-row indirect gathers.  (SWDGE
    indirect DMA tops out at 128 descriptors / ~1us per instruction,
    which lower-bounds any on-device row gather at ~4.3 ms/core here.)
  - Pass A (per tile): raw sums / second moments / value stats / semantic
    counts via fused multiply-accumulate (scalar_tensor_tensor accum_out)
    and strided tensor_reduce; centered coordinates are retained in SBUF.
  - Batched per-cluster math on [128, NT] tiles: closed-form symmetric 3x3
    eigenvalues (trig method via Arctan/Sin on the scalar engine),
    principal eigenvector via the spectral projector (A - w0)(A - w1),
    B = A / w2, dirwt = 1 - w1/w2, mode via argmax scan.
  - Pass B (per tile): orientation statistic sc = sum(t * |xc_perp|) from
    the retained centered coords; padded slots contribute a closed-form
    correction term.  Sign-flip + dirwt scaling, then 19 output planes are
    DMA'd out and decoded on the host.
"""

import sys

for _p in ("/opt/trn_rl_repo",):
    if _p not in sys.path:
        sys.path.insert(0, _p)

import numpy as np

N = 2_000_000
C = 32768
L = 256
N_CORES = 8
P = 128
NT = C // (P * N_CORES)  # 32 tiles per core
f32 = np.float32

_PI = float(np.pi)


def _host_prep(data, clust_idx, clust_len):
    data = np.asarray(data, dtype=f32)
    clust_idx = np.asarray(clust_idx).astype(np.int64)
    lens = np.asarray(clust_len).astype(np.int64)

    table = np.zeros((N + 1, 8), dtype=f32)
    table[:N, 0:3] = data[:, 0:3]
    table[:N, 3] = data[:, 4]
    sem = data[:, 5].astype(np.int32)
    for k in range(1, 5):
        table[:N, 3 + k] = (sem == k)

    order = np.argsort(lens, kind="stable")  # ascending length
    # global rank r: tile t = r // (P * N_CORES); slot s = r % (P * N_CORES)
    # core = s % N_CORES ; partition = s // N_CORES
    Lb = np.zeros(NT, dtype=np.int64)
    for t in range(NT):
        Lb[t] = lens[order[t * P * N_CORES:(t + 1) * P * N_CORES]].max()
    S = int(Lb.sum())

    # padded index matrix [C, L] with invalid slots -> N (zero row)
    ar = np.arange(L)[None, :]
    idx_pad = np.where(ar < lens[:, None], clust_idx, N)

    gst = np.zeros((N_CORES, P, S * 8), dtype=f32)
    nvecs = np.zeros((N_CORES, P, NT), dtype=f32)
    ids = np.zeros((N_CORES, NT, P), dtype=np.int64)
    off = 0
    for t in range(NT):
        base = t * P * N_CORES
        lb = int(Lb[t])
        for core in range(N_CORES):
            sel = order[base + core + N_CORES * np.arange(P)]
            ids[core, t] = sel
            nvecs[core, :, t] = lens[sel]
            gst[core, :, off * 8:(off + lb) * 8] = (
                table[idx_pad[sel, :lb]].reshape(P, lb * 8))
        off += lb
    return gst, nvecs, Lb, S, ids


def _build_program(Lb, S):
    import concourse.bass as bass
    import concourse.bacc as bacc
    import concourse.mybir as mybir
    from concourse.tile import TileContext

    dt = mybir.dt
    Alu = mybir.AluOpType
    Act = mybir.ActivationFunctionType

    nc = bacc.Bacc("TRN2", target_bir_lowering=False, debug=False,
                   enable_asserts=False)
    gst = nc.dram_tensor("gst", [P, S * 8], dt.float32, kind="ExternalInput")
    nvec_d = nc.dram_tensor("nvec", [P, NT], dt.float32, kind="ExternalInput")
    res = nc.dram_tensor("res", [P, 19 * NT], dt.float32, kind="ExternalOutput")

    TINY = 1e-30

    with TileContext(nc) as tc:
        with tc.tile_pool(name="ret", bufs=1) as ret, \
             tc.tile_pool(name="gp", bufs=4) as gp, \
             tc.tile_pool(name="sp", bufs=2) as sp:

            def full_tile(tag, k=1):
                return ret.tile([P, k * NT], dt.float32, tag=tag, name=tag)

            NV = full_tile("NV")
            RN = full_tile("RN")
            SUMS = full_tile("SUMS", 4)
            OH = full_tile("OH", 4)
            PROD = full_tile("PROD", 7)
            CEN = full_tile("CEN", 3)
            SCRAW = full_tile("SCRAW")
            MEANV = full_tile("MEANV"); STDV = full_tile("STDV")
            MODE = full_tile("MODE")
            B6 = full_tile("B6", 6)
            V3 = full_tile("V3", 3)

            nc.sync.dma_start(out=NV[:], in_=nvec_d[:, :])
            nc.vector.reciprocal(RN[:], NV[:])

            def tt(op, out, a, b):
                nc.vector.tensor_tensor(out=out, in0=a, in1=b, op=op)

            def ts(out, in0, s, op):
                nc.vector.tensor_scalar(out=out, in0=in0, scalar1=s,
                                        scalar2=None, op0=op)

            def stt(out, in0, s, op0, op1, in1, accum=None):
                nc.vector.scalar_tensor_tensor(out=out, in0=in0, scalar=s,
                                               in1=in1, op0=op0, op1=op1,
                                               accum_out=accum)

            def act(out, in_, func, bias=0.0, scale=1.0):
                nc.scalar.activation(out, in_, func, bias=bias, scale=scale)

            xcs = []
            offs = []
            off = 0
            for t in range(NT):
                offs.append(off)
                off += int(Lb[t])

            def load_and_pass_a(t):
                lb = int(Lb[t])
                G = gp.tile([P, lb * 8], dt.float32, tag="G", name=f"G{t}")
                nc.sync.dma_start(
                    out=G[:], in_=gst[:, offs[t] * 8:(offs[t] + lb) * 8])
                Gf = G[:].rearrange("p (l f) -> p f l", f=8)
                nc.vector.tensor_reduce(
                    out=SUMS[:].rearrange("p (f t) -> p f t", t=NT)[:, :, t],
                    in_=Gf[:, 0:4, :], axis=mybir.AxisListType.X, op=Alu.add)
                nc.vector.tensor_reduce(
                    out=OH[:].rearrange("p (f t) -> p f t", t=NT)[:, :, t],
                    in_=Gf[:, 4:8, :], axis=mybir.AxisListType.X, op=Alu.add)
                scratch = sp.tile([P, lb], dt.float32, tag="scr", name=f"scr{t}")
                pairs = [(0, 0), (0, 1), (0, 2), (1, 1), (1, 2), (2, 2), (3, 3)]
                for q, (i, j) in enumerate(pairs):
                    nc.vector.scalar_tensor_tensor(
                        out=scratch[:],
                        in0=Gf[:, i, :], scalar=1.0, in1=Gf[:, j, :],
                        op0=Alu.mult, op1=Alu.mult,
                        accum_out=PROD[:, q * NT + t:q * NT + t + 1])
                nc.vector.tensor_scalar(
                    out=CEN[:].rearrange("p (f t) -> p f t", t=NT)[:, :, t],
                    in0=SUMS[:].rearrange("p (f t) -> p f t", t=NT)[:, 0:3, t],
                    scalar1=RN[:, t:t + 1], scalar2=None, op0=Alu.mult)
                xc = ret.tile([P, 3 * lb], dt.float32, tag=f"xc{t}", name=f"xc{t}")
                for i in range(3):
                    nc.vector.tensor_scalar(
                        out=xc[:, i * lb:(i + 1) * lb],
                        in0=Gf[:, i, :],
                        scalar1=CEN[:, i * NT + t:i * NT + t + 1],
                        scalar2=None, op0=Alu.subtract)
                xcs.append(xc)

            def cluster_math():
                def tmp(tag, k=1):
                    return ret.tile([P, k * NT], dt.float32, tag=tag, name=tag)

                def sl(T, i):
                    return T[:, i * NT:(i + 1) * NT]

                A = tmp("A", 6)
                cmap = [(0, 0, 0), (1, 0, 1), (2, 0, 2), (3, 1, 1), (4, 1, 2),
                        (5, 2, 2)]
                SC1 = tmp("SC1")
                for q, i, j in cmap:
                    tt(Alu.mult, SC1[:], sl(CEN, i), sl(SUMS, j))
                    tt(Alu.subtract, sl(A, q), sl(PROD, q), SC1[:])

                # value stats
                VAR = tmp("VAR"); NM1 = tmp("NM1")
                tt(Alu.mult, MEANV[:], sl(SUMS, 3), RN[:])
                tt(Alu.mult, VAR[:], MEANV[:], sl(SUMS, 3))
                tt(Alu.subtract, VAR[:], sl(PROD, 6), VAR[:])
                ts(NM1[:], NV[:], 1.0, Alu.subtract)
                nc.vector.reciprocal(SC1[:], NM1[:])
                tt(Alu.mult, VAR[:], VAR[:], SC1[:])
                ts(VAR[:], VAR[:], 0.0, Alu.max)
                act(STDV[:], VAR[:], Act.Sqrt)

                BEST = tmp("BEST"); GT = tmp("GT"); KT = tmp("KT")

                # eigenvalues: trig closed form
                Q = tmp("Q"); P1 = tmp("P1"); P2 = tmp("P2"); PP = tmp("PP")
                RP = tmp("RP"); DET = tmp("DET"); RR = tmp("RR"); SS = tmp("SS")
                AT = tmp("AT"); PHI = tmp("PHI")
                W0 = tmp("W0"); W1 = tmp("W1"); W2 = tmp("W2"); RW2 = tmp("RW2")
                DIRWT = tmp("DIRWT")
                NB = tmp("NB", 6)

                tt(Alu.add, Q[:], sl(A, 0), sl(A, 3))
                tt(Alu.add, Q[:], Q[:], sl(A, 5))
                ts(Q[:], Q[:], 1.0 / 3.0, Alu.mult)

                tt(Alu.mult, P1[:], sl(A, 1), sl(A, 1))
                tt(Alu.mult, SC1[:], sl(A, 2), sl(A, 2))
                tt(Alu.add, P1[:], P1[:], SC1[:])
                tt(Alu.mult, SC1[:], sl(A, 4), sl(A, 4))
                tt(Alu.add, P1[:], P1[:], SC1[:])

                BD = tmp("BD", 3)
                tt(Alu.subtract, sl(BD, 0), sl(A, 0), Q[:])
                tt(Alu.subtract, sl(BD, 1), sl(A, 3), Q[:])
                tt(Alu.subtract, sl(BD, 2), sl(A, 5), Q[:])
                tt(Alu.mult, P2[:], sl(BD, 0), sl(BD, 0))
                tt(Alu.mult, SC1[:], sl(BD, 1), sl(BD, 1))
                tt(Alu.add, P2[:], P2[:], SC1[:])
                tt(Alu.mult, SC1[:], sl(BD, 2), sl(BD, 2))
                tt(Alu.add, P2[:], P2[:], SC1[:])
                stt(P2[:], P1[:], 2.0, Alu.mult, Alu.add, P2[:])
                ts(PP[:], P2[:], 1.0 / 6.0, Alu.mult)
                act(PP[:], PP[:], Act.Sqrt)
                ts(SC1[:], PP[:], TINY, Alu.max)
                nc.vector.reciprocal(RP[:], SC1[:])

                tt(Alu.mult, sl(NB, 0), sl(BD, 0), RP[:])
                tt(Alu.mult, sl(NB, 1), sl(A, 1), RP[:])
                tt(Alu.mult, sl(NB, 2), sl(A, 2), RP[:])
                tt(Alu.mult, sl(NB, 3), sl(BD, 1), RP[:])
                tt(Alu.mult, sl(NB, 4), sl(A, 4), RP[:])
                tt(Alu.mult, sl(NB, 5), sl(BD, 2), RP[:])

                SC2 = tmp("SC2"); SC3 = tmp("SC3")
                tt(Alu.mult, SC1[:], sl(NB, 3), sl(NB, 5))
                tt(Alu.mult, SC2[:], sl(NB, 4), sl(NB, 4))
                tt(Alu.subtract, SC1[:], SC1[:], SC2[:])
                tt(Alu.mult, DET[:], sl(NB, 0), SC1[:])
                tt(Alu.mult, SC1[:], sl(NB, 1), sl(NB, 5))
                tt(Alu.mult, SC2[:], sl(NB, 4), sl(NB, 2))
                tt(Alu.subtract, SC1[:], SC1[:], SC2[:])
                tt(Alu.mult, SC1[:], sl(NB, 1), SC1[:])
                tt(Alu.subtract, DET[:], DET[:], SC1[:])
                tt(Alu.mult, SC1[:], sl(NB, 1), sl(NB, 4))
                tt(Alu.mult, SC2[:], sl(NB, 3), sl(NB, 2))
                tt(Alu.subtract, SC1[:], SC1[:], SC2[:])
                tt(Alu.mult, SC1[:], sl(NB, 2), SC1[:])
                tt(Alu.add, DET[:], DET[:], SC1[:])

                ts(RR[:], DET[:], 0.5, Alu.mult)
                ts(RR[:], RR[:], -1.0, Alu.max)
                ts(RR[:], RR[:], 1.0, Alu.min)
                tt(Alu.mult, SS[:], RR[:], RR[:])
                nc.vector.tensor_scalar(out=SS[:], in0=SS[:], scalar1=-1.0,
                                        scalar2=1.0, op0=Alu.mult, op1=Alu.add)
                ts(SS[:], SS[:], 0.0, Alu.max)
                act(SS[:], SS[:], Act.Sqrt)
                UA = tmp("UA"); UB = tmp("UB")
                ts(SC1[:], RR[:], -1.0, Alu.mult)
                tt(Alu.max, SC1[:], SC1[:], RR[:])
                ts(SS[:], SS[:], TINY, Alu.max)
                nc.vector.reciprocal(SC2[:], SS[:])
                tt(Alu.mult, UA[:], SC1[:], SC2[:])
                ts(SC1[:], UA[:], TINY, Alu.max)
                nc.vector.reciprocal(UB[:], SC1[:])
                tt(Alu.min, SC2[:], UA[:], UB[:])
                act(SC2[:], SC2[:], Act.Arctan)
                ts(SC1[:], UA[:], 1.0, Alu.is_gt)
                nc.vector.tensor_scalar(out=SC3[:], in0=SC2[:], scalar1=-2.0,
                                        scalar2=_PI / 2.0, op0=Alu.mult,
                                        op1=Alu.add)
                tt(Alu.mult, SC3[:], SC3[:], SC1[:])
                tt(Alu.add, SC2[:], SC2[:], SC3[:])
                ts(SC3[:], RR[:], 0.0, Alu.is_lt)
                nc.vector.tensor_scalar(out=SC3[:], in0=SC3[:], scalar1=-2.0,
                                        scalar2=1.0, op0=Alu.mult, op1=Alu.add)
                tt(Alu.mult, AT[:], SC2[:], SC3[:])
                nc.vector.tensor_scalar(out=PHI[:], in0=AT[:],
                                        scalar1=-1.0 / 3.0,
                                        scalar2=_PI / 6.0 + _PI / 2.0,
                                        op0=Alu.mult, op1=Alu.add)
                act(SC1[:], PHI[:], Act.Sin)
                tt(Alu.mult, SC1[:], SC1[:], PP[:])
                stt(W2[:], SC1[:], 2.0, Alu.mult, Alu.add, Q[:])
                nc.vector.tensor_scalar(out=PHI[:], in0=AT[:],
                                        scalar1=-1.0 / 3.0,
                                        scalar2=_PI / 6.0 + _PI / 6.0,
                                        op0=Alu.mult, op1=Alu.add)
                act(SC1[:], PHI[:], Act.Sin)
                tt(Alu.mult, SC1[:], SC1[:], PP[:])
                stt(W0[:], SC1[:], -2.0, Alu.mult, Alu.add, Q[:])
                ts(SC1[:], Q[:], 3.0, Alu.mult)
                tt(Alu.subtract, W1[:], SC1[:], W0[:])
                tt(Alu.subtract, W1[:], W1[:], W2[:])

                ts(SC1[:], W2[:], TINY, Alu.max)
                nc.vector.reciprocal(RW2[:], SC1[:])
                tt(Alu.mult, DIRWT[:], W1[:], RW2[:])
                nc.vector.tensor_scalar(out=DIRWT[:], in0=DIRWT[:],
                                        scalar1=-1.0, scalar2=1.0,
                                        op0=Alu.mult, op1=Alu.add)
                for q in range(6):
                    tt(Alu.mult, sl(B6, q), sl(A, q), RW2[:])

                CD = tmp("CD", 3)
                DD = tmp("DD", 3)
                for qi, ai in enumerate((0, 3, 5)):
                    tt(Alu.subtract, sl(CD, qi), sl(A, ai), W0[:])
                    tt(Alu.subtract, sl(DD, qi), sl(A, ai), W1[:])
                M9 = tmp("M9", 9)

                def mcol(colq, dv):
                    crow = [(sl(CD, 0), sl(A, 1), sl(A, 2)),
                            (sl(A, 1), sl(CD, 1), sl(A, 4)),
                            (sl(A, 2), sl(A, 4), sl(CD, 2))]
                    for r in range(3):
                        a0, a1, a2 = crow[r]
                        tt(Alu.mult, SC1[:], a0, dv[0])
                        tt(Alu.mult, SC2[:], a1, dv[1])
                        tt(Alu.add, SC1[:], SC1[:], SC2[:])
                        tt(Alu.mult, SC2[:], a2, dv[2])
                        tt(Alu.add, sl(M9, colq * 3 + r), SC1[:], SC2[:])

                mcol(0, (sl(DD, 0), sl(A, 1), sl(A, 2)))
                mcol(1, (sl(A, 1), sl(DD, 1), sl(A, 4)))
                mcol(2, (sl(A, 2), sl(A, 4), sl(DD, 2)))

                CN = tmp("CN", 3)
                for j in range(3):
                    tt(Alu.mult, sl(CN, j), sl(M9, j * 3), sl(M9, j * 3))
                    tt(Alu.mult, SC1[:], sl(M9, j * 3 + 1), sl(M9, j * 3 + 1))
                    tt(Alu.add, sl(CN, j), sl(CN, j), SC1[:])
                    tt(Alu.mult, SC1[:], sl(M9, j * 3 + 2), sl(M9, j * 3 + 2))
                    tt(Alu.add, sl(CN, j), sl(CN, j), SC1[:])
                NBEST = tmp("NBEST")
                for i in range(3):
                    nc.vector.tensor_copy(out=sl(V3, i), in_=sl(M9, i))
                nc.vector.tensor_copy(out=NBEST[:], in_=sl(CN, 0))
                for j in (1, 2):
                    tt(Alu.is_gt, GT[:], sl(CN, j), NBEST[:])
                    for i in range(3):
                        tt(Alu.subtract, SC1[:], sl(M9, j * 3 + i), sl(V3, i))
                        tt(Alu.mult, SC1[:], SC1[:], GT[:])
                        tt(Alu.add, sl(V3, i), sl(V3, i), SC1[:])
                    tt(Alu.max, NBEST[:], NBEST[:], sl(CN, j))
                ts(SC1[:], NBEST[:], 1e-37, Alu.max)
                act(SC2[:], SC1[:], Act.Sqrt)
                nc.vector.reciprocal(SC2[:], SC2[:])
                for i in range(3):
                    tt(Alu.mult, sl(V3, i), sl(V3, i), SC2[:])

                # mode of semantic class (ties -> smallest)
                tt(Alu.subtract, BEST[:], NV[:], sl(OH, 0))
                for k in (1, 2, 3):
                    tt(Alu.subtract, BEST[:], BEST[:], sl(OH, k))
                nc.vector.memset(MODE[:], 0.0)
                for k in range(1, 5):
                    ck = sl(OH, k - 1)
                    tt(Alu.is_gt, GT[:], ck, BEST[:])
                    nc.vector.tensor_scalar(out=KT[:], in0=MODE[:],
                                            scalar1=-1.0, scalar2=float(k),
                                            op0=Alu.mult, op1=Alu.add)
                    tt(Alu.mult, KT[:], KT[:], GT[:])
                    tt(Alu.add, MODE[:], MODE[:], KT[:])
                    tt(Alu.max, BEST[:], BEST[:], ck)
                return DIRWT

            def pass_b(t):
                lb = int(Lb[t])
                xc = xcs[t]
                xcx = xc[:, 0:lb]; xcy = xc[:, lb:2 * lb]
                xcz = xc[:, 2 * lb:3 * lb]
                T = sp.tile([P, lb], dt.float32, tag="T", name=f"T{t}")
                S2 = sp.tile([P, lb], dt.float32, tag="S2", name=f"S2_{t}")
                S2b = sp.tile([P, lb], dt.float32, tag="S2b", name=f"S2b{t}")
                R = sp.tile([P, lb], dt.float32, tag="R", name=f"R{t}")
                nc.vector.tensor_scalar(out=T[:], in0=xcx,
                                        scalar1=V3[:, 0 * NT + t:0 * NT + t + 1],
                                        scalar2=None, op0=Alu.mult)
                stt(T[:], xcy, V3[:, 1 * NT + t:1 * NT + t + 1],
                    Alu.mult, Alu.add, T[:])
                stt(T[:], xcz, V3[:, 2 * NT + t:2 * NT + t + 1],
                    Alu.mult, Alu.add, T[:])
                stt(S2[:], xcx, 1.0, Alu.mult, Alu.mult, xcx)
                stt(S2b[:], xcy, 1.0, Alu.mult, Alu.mult, xcy)
                tt(Alu.add, S2[:], S2[:], S2b[:])
                stt(S2b[:], xcz, 1.0, Alu.mult, Alu.mult, xcz)
                tt(Alu.add, S2[:], S2[:], S2b[:])
                stt(S2b[:], T[:], 1.0, Alu.mult, Alu.mult, T[:])
                tt(Alu.subtract, S2[:], S2[:], S2b[:])
                ts(S2[:], S2[:], 0.0, Alu.max)
                act(R[:], S2[:], Act.Sqrt)
                stt(S2b[:], T[:], 1.0, Alu.mult, Alu.mult, R[:],
                    accum=SCRAW[:, t:t + 1])

            def sign_phase(DIRWT):
                def tmp(tag, k=1):
                    return ret.tile([P, k * NT], dt.float32, tag=tag, name=tag)

                def sl(T, i):
                    return T[:, i * NT:(i + 1) * NT]

                T0 = tmp("T0"); CC = tmp("CC"); R0 = tmp("R0")
                SCV = tmp("SCV"); FAC = tmp("FAC"); SC9 = tmp("SC9")
                GT9 = tmp("GT9"); NPAD = tmp("NPAD")
                tt(Alu.mult, T0[:], sl(CEN, 0), sl(V3, 0))
                tt(Alu.mult, SC9[:], sl(CEN, 1), sl(V3, 1))
                tt(Alu.add, T0[:], T0[:], SC9[:])
                tt(Alu.mult, SC9[:], sl(CEN, 2), sl(V3, 2))
                tt(Alu.add, T0[:], T0[:], SC9[:])
                ts(T0[:], T0[:], -1.0, Alu.mult)
                tt(Alu.mult, CC[:], sl(CEN, 0), sl(CEN, 0))
                tt(Alu.mult, SC9[:], sl(CEN, 1), sl(CEN, 1))
                tt(Alu.add, CC[:], CC[:], SC9[:])
                tt(Alu.mult, SC9[:], sl(CEN, 2), sl(CEN, 2))
                tt(Alu.add, CC[:], CC[:], SC9[:])
                tt(Alu.mult, SC9[:], T0[:], T0[:])
                tt(Alu.subtract, R0[:], CC[:], SC9[:])
                ts(R0[:], R0[:], 0.0, Alu.max)
                act(R0[:], R0[:], Act.Sqrt)
                for t in range(NT):
                    nc.vector.tensor_scalar(
                        out=NPAD[:, t:t + 1],
                        in0=NV[:, t:t + 1], scalar1=-1.0,
                        scalar2=float(int(Lb[t])), op0=Alu.mult, op1=Alu.add)
                tt(Alu.mult, SC9[:], T0[:], R0[:])
                tt(Alu.mult, SC9[:], SC9[:], NPAD[:])
                tt(Alu.subtract, SCV[:], SCRAW[:], SC9[:])
                ts(GT9[:], SCV[:], 0.0, Alu.is_lt)
                nc.vector.tensor_scalar(out=GT9[:], in0=GT9[:], scalar1=-2.0,
                                        scalar2=1.0, op0=Alu.mult, op1=Alu.add)
                tt(Alu.mult, FAC[:], DIRWT[:], GT9[:])
                for i in range(3):
                    tt(Alu.mult, sl(V3, i), sl(V3, i), FAC[:])
                for j, pl in [(0, sl(CEN, 0)), (1, sl(CEN, 1)), (2, sl(CEN, 2)),
                              (3, sl(B6, 0)), (4, sl(B6, 1)), (5, sl(B6, 2)),
                              (6, sl(B6, 1)), (7, sl(B6, 3)), (8, sl(B6, 4)),
                              (9, sl(B6, 2)), (10, sl(B6, 4)), (11, sl(B6, 5)),
                              (12, sl(V3, 0)), (13, sl(V3, 1)), (14, sl(V3, 2)),
                              (15, NV[:]), (16, MEANV[:]), (17, STDV[:]),
                              (18, MODE[:])]:
                    nc.sync.dma_start(out=res[:, j * NT:(j + 1) * NT], in_=pl)

            for t in range(NT):
                load_and_pass_a(t)
            DIRWT = cluster_math()
            for t in range(NT):
                pass_b(t)
            sign_phase(DIRWT)

    nc.compile()
    return nc


_cache = {}
_last = None


def kernel(data, clust_idx, clust_len):
    global N, C, L, NT
    data = np.asarray(data)
    clust_idx = np.asarray(clust_idx)
    N = int(data.shape[0])
    C, L = int(clust_idx.shape[0]), int(clust_idx.shape[1])
    assert C % (P * N_CORES) == 0, f"cluster count {C} not divisible by {P * N_CORES}"
    NT = C // (P * N_CORES)
    gst, nvecs, Lb, S, ids = _host_prep(data, clust_idx, clust_len)

    key = tuple(int(x) for x in Lb)
    if key not in _cache:
        _cache[key] = _build_program(Lb, S)
    nc = _cache[key]

    from concourse.bass_utils import run_bass_kernel_spmd
    in_maps = [{"gst": gst[c], "nvec": nvecs[c]} for c in range(N_CORES)]
    global _last
    _last = (nc, in_maps)
    res = run_bass_kernel_spmd(nc, in_maps, list(range(N_CORES)))

    out = np.zeros((C, 19), dtype=f32)
    for core in range(N_CORES):
        r = res.results[core]["res"].reshape(P, 19, NT)
        for t in range(NT):
            out[ids[core, t]] = r[:, :, t]
    return out


# revision 9
# speedup vs baseline: 1.3092x; 1.3092x over previous
"""Trainium2 Bass kernel for nn_ClustGeoNodeEncoder (segment_reduce).

Strategy (data-parallel over the cluster axis, per the sharding hint):
  - Host packs the voxel features as 8-f32 rows: x, y, z, value,
    onehot(sem==1..4); count of sem==0 is recovered as n - sum(oh1..4).
  - Clusters are sorted by length and dealt round-robin to the 8 cores so
    every core compiles the same program (SPMD): 32 tiles x 128 clusters
    per core, tile t padded to Lb[t] = max length in its global rank range.
  - The host materializes the per-core dense element stream
    gst[P, S*8] (cluster p's elements for tile t at columns
    off_t*8:(off_t+lb)*8, padded slots all-zero).  The device streams it
    with plain contiguous DMAs -- no per-row indirect gathers.  (SWDGE
    indirect DMA tops out at 128 descriptors / ~1us per instruction,
    which lower-bounds any on-device row gather at ~4.3 ms/core here.)
  - Elementwise work is split between the DVE (vector) engine and the
    Scalar (ACT) engine: ACT does all squared terms (Act.Square with
    accum_out for the diagonal moments), the centering (Identity with a
    per-partition bias of -center), and the square roots.
  - Pass A (per tile): raw sums / second moments / value stats / semantic
    counts via strided tensor_reduce and fused multiply-accumulate;
    centered coordinates are retained in SBUF.
  - Batched per-cluster math on [128, NT] tiles: closed-form symmetric 3x3
    eigenvalues (trig method via Arctan/Sin on the scalar engine),
    principal eigenvector via the spectral projector (A - w0)(A - w1),
    B = A / w2, dirwt = 1 - w1/w2, mode via argmax scan.
  - Pass B (per tile): orientation statistic sc = sum(t * |xc_perp|) from
    the retained centered coords; padded slots contribute a closed-form
    correction term.  Sign-flip + dirwt scaling, then 19 output planes are
    DMA'd out and decoded on the host.
"""

import sys

for _p in ("/opt/trn_rl_repo",):
    if _p not in sys.path:
        sys.path.insert(0, _p)

import numpy as np

N = 2_000_000
C = 32768
L = 256
N_CORES = 8
P = 128
NT = C // (P * N_CORES)  # 32 tiles per core
f32 = np.float32

_PI = float(np.pi)


def _host_prep(data, clust_idx, clust_len):
    data = np.asarray(data, dtype=f32)
    clust_idx = np.asarray(clust_idx).astype(np.int64)
    lens = np.asarray(clust_len).astype(np.int64)

    table = np.zeros((N + 1, 8), dtype=f32)
    table[:N, 0:3] = data[:, 0:3]
    table[:N, 3] = data[:, 4]
    sem = data[:, 5].astype(np.int32)
    for k in range(1, 5):
        table[:N, 3 + k] = (sem == k)

    order = np.argsort(lens, kind="stable")  # ascending length
    # global rank r: tile t = r // (P * N_CORES); slot s = r % (P * N_CORES)
    # core = s % N_CORES ; partition = s // N_CORES
    Lb = np.zeros(NT, dtype=np.int64)
    for t in range(NT):
        Lb[t] = lens[order[t * P * N_CORES:(t + 1) * P * N_CORES]].max()
    S = int(Lb.sum())

    # padded index matrix [C, L] with invalid slots -> N (zero row)
    ar = np.arange(L)[None, :]
    idx_pad = np.where(ar < lens[:, None], clust_idx, N)

    gst = np.zeros((N_CORES, P, S * 8), dtype=f32)
    nvecs = np.zeros((N_CORES, P, NT), dtype=f32)
    ids = np.zeros((N_CORES, NT, P), dtype=np.int64)
    off = 0
    for t in range(NT):
        base = t * P * N_CORES
        lb = int(Lb[t])
        for core in range(N_CORES):
            sel = order[base + core + N_CORES * np.arange(P)]
            ids[core, t] = sel
            nvecs[core, :, t] = lens[sel]
            gst[core, :, off * 8:(off + lb) * 8] = (
                table[idx_pad[sel, :lb]].reshape(P, lb * 8))
        off += lb
    return gst, nvecs, Lb, S, ids


def _build_program(Lb, S):
    import concourse.bass as bass
    import concourse.bacc as bacc
    import concourse.mybir as mybir
    from concourse.tile import TileContext

    dt = mybir.dt
    Alu = mybir.AluOpType
    Act = mybir.ActivationFunctionType

    nc = bacc.Bacc("TRN2", target_bir_lowering=False, debug=False,
                   enable_asserts=False)
    gst = nc.dram_tensor("gst", [P, S * 8], dt.float32, kind="ExternalInput")
    nvec_d = nc.dram_tensor("nvec", [P, NT], dt.float32, kind="ExternalInput")
    res = nc.dram_tensor("res", [P, 19 * NT], dt.float32, kind="ExternalOutput")

    TINY = 1e-30

    with TileContext(nc) as tc:
        with tc.tile_pool(name="ret", bufs=1) as ret, \
             tc.tile_pool(name="gp", bufs=4) as gp, \
             tc.tile_pool(name="sp", bufs=2) as sp, \
             tc.tile_pool(name="sq", bufs=2) as sq:

            def full_tile(tag, k=1):
                return ret.tile([P, k * NT], dt.float32, tag=tag, name=tag)

            NV = full_tile("NV")
            RN = full_tile("RN")
            SUMS = full_tile("SUMS", 4)
            OH = full_tile("OH", 4)
            PROD = full_tile("PROD", 7)
            CEN = full_tile("CEN", 3)
            NCEN = full_tile("NCEN", 3)
            SCRAW = full_tile("SCRAW")
            MEANV = full_tile("MEANV"); STDV = full_tile("STDV")
            MODE = full_tile("MODE")
            B6 = full_tile("B6", 6)
            V3 = full_tile("V3", 3)

            nc.sync.dma_start(out=NV[:], in_=nvec_d[:, :])
            nc.vector.reciprocal(RN[:], NV[:])

            def tt(op, out, a, b):
                nc.vector.tensor_tensor(out=out, in0=a, in1=b, op=op)

            def ts(out, in0, s, op):
                nc.vector.tensor_scalar(out=out, in0=in0, scalar1=s,
                                        scalar2=None, op0=op)

            def stt(out, in0, s, op0, op1, in1, accum=None):
                nc.vector.scalar_tensor_tensor(out=out, in0=in0, scalar=s,
                                               in1=in1, op0=op0, op1=op1,
                                               accum_out=accum)

            def act(out, in_, func, bias=0.0, scale=1.0, accum=None):
                nc.scalar.activation(out, in_, func, bias=bias, scale=scale,
                                     accum_out=accum)

            xcs = []
            offs = []
            off = 0
            for t in range(NT):
                offs.append(off)
                off += int(Lb[t])

            def load_and_pass_a(t):
                lb = int(Lb[t])
                G = gp.tile([P, lb * 8], dt.float32, tag="G", name=f"G{t}")
                nc.sync.dma_start(
                    out=G[:], in_=gst[:, offs[t] * 8:(offs[t] + lb) * 8])
                Gf = G[:].rearrange("p (l f) -> p f l", f=8)
                nc.vector.tensor_reduce(
                    out=SUMS[:].rearrange("p (f t) -> p f t", t=NT)[:, :, t],
                    in_=Gf[:, 0:4, :], axis=mybir.AxisListType.X, op=Alu.add)
                nc.vector.tensor_reduce(
                    out=OH[:].rearrange("p (f t) -> p f t", t=NT)[:, :, t],
                    in_=Gf[:, 4:8, :], axis=mybir.AxisListType.X, op=Alu.add)
                # diagonal moments xx, yy, zz, vv on the scalar engine
                sqs = sq.tile([P, 4 * lb], dt.float32, tag="sqs", name=f"sqs{t}")
                for q, i in ((0, 0), (3, 1), (5, 2), (6, 3)):
                    act(sqs[:, i * lb:(i + 1) * lb], Gf[:, i, :], Act.Square,
                        accum=PROD[:, q * NT + t:q * NT + t + 1])
                # cross moments xy, xz, yz on DVE
                scratch = sp.tile([P, lb], dt.float32, tag="scr", name=f"scr{t}")
                for q, (i, j) in ((1, (0, 1)), (2, (0, 2)), (4, (1, 2))):
                    nc.vector.scalar_tensor_tensor(
                        out=scratch[:],
                        in0=Gf[:, i, :], scalar=1.0, in1=Gf[:, j, :],
                        op0=Alu.mult, op1=Alu.mult,
                        accum_out=PROD[:, q * NT + t:q * NT + t + 1])
                # center and its negation (bias for the ACT centering)
                nc.vector.tensor_scalar(
                    out=NCEN[:].rearrange("p (f t) -> p f t", t=NT)[:, :, t],
                    in0=SUMS[:].rearrange("p (f t) -> p f t", t=NT)[:, 0:3, t],
                    scalar1=RN[:, t:t + 1], scalar2=-1.0,
                    op0=Alu.mult, op1=Alu.mult)
                ts(CEN[:].rearrange("p (f t) -> p f t", t=NT)[:, :, t],
                   NCEN[:].rearrange("p (f t) -> p f t", t=NT)[:, :, t],
                   -1.0, Alu.mult)
                # centered coords on the scalar engine: xc = x + (-c)
                xc = ret.tile([P, 3 * lb], dt.float32, tag=f"xc{t}", name=f"xc{t}")
                for i in range(3):
                    act(xc[:, i * lb:(i + 1) * lb], Gf[:, i, :], Act.Identity,
                        bias=NCEN[:, i * NT + t:i * NT + t + 1])
                xcs.append(xc)

            def cluster_math():
                def tmp(tag, k=1):
                    return ret.tile([P, k * NT], dt.float32, tag=tag, name=tag)

                def sl(T, i):
                    return T[:, i * NT:(i + 1) * NT]

                A = tmp("A", 6)
                cmap = [(0, 0, 0), (1, 0, 1), (2, 0, 2), (3, 1, 1), (4, 1, 2),
                        (5, 2, 2)]
                SC1 = tmp("SC1")
                for q, i, j in cmap:
                    tt(Alu.mult, SC1[:], sl(CEN, i), sl(SUMS, j))
                    tt(Alu.subtract, sl(A, q), sl(PROD, q), SC1[:])

                # value stats
                VAR = tmp("VAR"); NM1 = tmp("NM1")
                tt(Alu.mult, MEANV[:], sl(SUMS, 3), RN[:])
                tt(Alu.mult, VAR[:], MEANV[:], sl(SUMS, 3))
                tt(Alu.subtract, VAR[:], sl(PROD, 6), VAR[:])
                ts(NM1[:], NV[:], 1.0, Alu.subtract)
                nc.vector.reciprocal(SC1[:], NM1[:])
                tt(Alu.mult, VAR[:], VAR[:], SC1[:])
                ts(VAR[:], VAR[:], 0.0, Alu.max)
                act(STDV[:], VAR[:], Act.Sqrt)

                BEST = tmp("BEST"); GT = tmp("GT"); KT = tmp("KT")

                # eigenvalues: trig closed form
                Q = tmp("Q"); P1 = tmp("P1"); P2 = tmp("P2"); PP = tmp("PP")
                RP = tmp("RP"); DET = tmp("DET"); RR = tmp("RR"); SS = tmp("SS")
                AT = tmp("AT"); PHI = tmp("PHI")
                W0 = tmp("W0"); W1 = tmp("W1"); W2 = tmp("W2"); RW2 = tmp("RW2")
                DIRWT = tmp("DIRWT")
                NB = tmp("NB", 6)

                tt(Alu.add, Q[:], sl(A, 0), sl(A, 3))
                tt(Alu.add, Q[:], Q[:], sl(A, 5))
                ts(Q[:], Q[:], 1.0 / 3.0, Alu.mult)

                tt(Alu.mult, P1[:], sl(A, 1), sl(A, 1))
                tt(Alu.mult, SC1[:], sl(A, 2), sl(A, 2))
                tt(Alu.add, P1[:], P1[:], SC1[:])
                tt(Alu.mult, SC1[:], sl(A, 4), sl(A, 4))
                tt(Alu.add, P1[:], P1[:], SC1[:])

                BD = tmp("BD", 3)
                tt(Alu.subtract, sl(BD, 0), sl(A, 0), Q[:])
                tt(Alu.subtract, sl(BD, 1), sl(A, 3), Q[:])
                tt(Alu.subtract, sl(BD, 2), sl(A, 5), Q[:])
                tt(Alu.mult, P2[:], sl(BD, 0), sl(BD, 0))
                tt(Alu.mult, SC1[:], sl(BD, 1), sl(BD, 1))
                tt(Alu.add, P2[:], P2[:], SC1[:])
                tt(Alu.mult, SC1[:], sl(BD, 2), sl(BD, 2))
                tt(Alu.add, P2[:], P2[:], SC1[:])
                stt(P2[:], P1[:], 2.0, Alu.mult, Alu.add, P2[:])
                ts(PP[:], P2[:], 1.0 / 6.0, Alu.mult)
                act(PP[:], PP[:], Act.Sqrt)
                ts(SC1[:], PP[:], TINY, Alu.max)
                nc.vector.reciprocal(RP[:], SC1[:])

                tt(Alu.mult, sl(NB, 0), sl(BD, 0), RP[:])
                tt(Alu.mult, sl(NB, 1), sl(A, 1), RP[:])
                tt(Alu.mult, sl(NB, 2), sl(A, 2), RP[:])
                tt(Alu.mult, sl(NB, 3), sl(BD, 1), RP[:])
                tt(Alu.mult, sl(NB, 4), sl(A, 4), RP[:])
                tt(Alu.mult, sl(NB, 5), sl(BD, 2), RP[:])

                SC2 = tmp("SC2"); SC3 = tmp("SC3")
                tt(Alu.mult, SC1[:], sl(NB, 3), sl(NB, 5))
                tt(Alu.mult, SC2[:], sl(NB, 4), sl(NB, 4))
                tt(Alu.subtract, SC1[:], SC1[:], SC2[:])
                tt(Alu.mult, DET[:], sl(NB, 0), SC1[:])
                tt(Alu.mult, SC1[:], sl(NB, 1), sl(NB, 5))
                tt(Alu.mult, SC2[:], sl(NB, 4), sl(NB, 2))
                tt(Alu.subtract, SC1[:], SC1[:], SC2[:])
                tt(Alu.mult, SC1[:], sl(NB, 1), SC1[:])
                tt(Alu.subtract, DET[:], DET[:], SC1[:])
                tt(Alu.mult, SC1[:], sl(NB, 1), sl(NB, 4))
                tt(Alu.mult, SC2[:], sl(NB, 3), sl(NB, 2))
                tt(Alu.subtract, SC1[:], SC1[:], SC2[:])
                tt(Alu.mult, SC1[:], sl(NB, 2), SC1[:])
                tt(Alu.add, DET[:], DET[:], SC1[:])

                ts(RR[:], DET[:], 0.5, Alu.mult)
                ts(RR[:], RR[:], -1.0, Alu.max)
                ts(RR[:], RR[:], 1.0, Alu.min)
                tt(Alu.mult, SS[:], RR[:], RR[:])
                nc.vector.tensor_scalar(out=SS[:], in0=SS[:], scalar1=-1.0,
                                        scalar2=1.0, op0=Alu.mult, op1=Alu.add)
                ts(SS[:], SS[:], 0.0, Alu.max)
                act(SS[:], SS[:], Act.Sqrt)
                UA = tmp("UA"); UB = tmp("UB")
                ts(SC1[:], RR[:], -1.0, Alu.mult)
                tt(Alu.max, SC1[:], SC1[:], RR[:])
                ts(SS[:], SS[:], TINY, Alu.max)
                nc.vector.reciprocal(SC2[:], SS[:])
                tt(Alu.mult, UA[:], SC1[:], SC2[:])
                ts(SC1[:], UA[:], TINY, Alu.max)
                nc.vector.reciprocal(UB[:], SC1[:])
                tt(Alu.min, SC2[:], UA[:], UB[:])
                act(SC2[:], SC2[:], Act.Arctan)
                ts(SC1[:], UA[:], 1.0, Alu.is_gt)
                nc.vector.tensor_scalar(out=SC3[:], in0=SC2[:], scalar1=-2.0,
                                        scalar2=_PI / 2.0, op0=Alu.mult,
                                        op1=Alu.add)
                tt(Alu.mult, SC3[:], SC3[:], SC1[:])
                tt(Alu.add, SC2[:], SC2[:], SC3[:])
                ts(SC3[:], RR[:], 0.0, Alu.is_lt)
                nc.vector.tensor_scalar(out=SC3[:], in0=SC3[:], scalar1=-2.0,
                                        scalar2=1.0, op0=Alu.mult, op1=Alu.add)
                tt(Alu.mult, AT[:], SC2[:], SC3[:])
                nc.vector.tensor_scalar(out=PHI[:], in0=AT[:],
                                        scalar1=-1.0 / 3.0,
                                        scalar2=_PI / 6.0 + _PI / 2.0,
                                        op0=Alu.mult, op1=Alu.add)
                act(SC1[:], PHI[:], Act.Sin)
                tt(Alu.mult, SC1[:], SC1[:], PP[:])
                stt(W2[:], SC1[:], 2.0, Alu.mult, Alu.add, Q[:])
                nc.vector.tensor_scalar(out=PHI[:], in0=AT[:],
                                        scalar1=-1.0 / 3.0,
                                        scalar2=_PI / 6.0 + _PI / 6.0,
                                        op0=Alu.mult, op1=Alu.add)
                act(SC1[:], PHI[:], Act.Sin)
                tt(Alu.mult, SC1[:], SC1[:], PP[:])
                stt(W0[:], SC1[:], -2.0, Alu.mult, Alu.add, Q[:])
                ts(SC1[:], Q[:], 3.0, Alu.mult)
                tt(Alu.subtract, W1[:], SC1[:], W0[:])
                tt(Alu.subtract, W1[:], W1[:], W2[:])

                ts(SC1[:], W2[:], TINY, Alu.max)
                nc.vector.reciprocal(RW2[:], SC1[:])
                tt(Alu.mult, DIRWT[:], W1[:], RW2[:])
                nc.vector.tensor_scalar(out=DIRWT[:], in0=DIRWT[:],
                                        scalar1=-1.0, scalar2=1.0,
                                        op0=Alu.mult, op1=Alu.add)
                for q in range(6):
                    tt(Alu.mult, sl(B6, q), sl(A, q), RW2[:])

                CD = tmp("CD", 3)
                DD = tmp("DD", 3)
                for qi, ai in enumerate((0, 3, 5)):
                    tt(Alu.subtract, sl(CD, qi), sl(A, ai), W0[:])
                    tt(Alu.subtract, sl(DD, qi), sl(A, ai), W1[:])
                M9 = tmp("M9", 9)

                def mcol(colq, dv):
                    crow = [(sl(CD, 0), sl(A, 1), sl(A, 2)),
                            (sl(A, 1), sl(CD, 1), sl(A, 4)),
                            (sl(A, 2), sl(A, 4), sl(CD, 2))]
                    for r in range(3):
                        a0, a1, a2 = crow[r]
                        tt(Alu.mult, SC1[:], a0, dv[0])
                        tt(Alu.mult, SC2[:], a1, dv[1])
                        tt(Alu.add, SC1[:], SC1[:], SC2[:])
                        tt(Alu.mult, SC2[:], a2, dv[2])
                        tt(Alu.add, sl(M9, colq * 3 + r), SC1[:], SC2[:])

                mcol(0, (sl(DD, 0), sl(A, 1), sl(A, 2)))
                mcol(1, (sl(A, 1), sl(DD, 1), sl(A, 4)))
                mcol(2, (sl(A, 2), sl(A, 4), sl(DD, 2)))

                CN = tmp("CN", 3)
                for j in range(3):
                    tt(Alu.mult, sl(CN, j), sl(M9, j * 3), sl(M9, j * 3))
                    tt(Alu.mult, SC1[:], sl(M9, j * 3 + 1), sl(M9, j * 3 + 1))
                    tt(Alu.add, sl(CN, j), sl(CN, j), SC1[:])
                    tt(Alu.mult, SC1[:], sl(M9, j * 3 + 2), sl(M9, j * 3 + 2))
                    tt(Alu.add, sl(CN, j), sl(CN, j), SC1[:])
                NBEST = tmp("NBEST")
                for i in range(3):
                    nc.vector.tensor_copy(out=sl(V3, i), in_=sl(M9, i))
                nc.vector.tensor_copy(out=NBEST[:], in_=sl(CN, 0))
                for j in (1, 2):
                    tt(Alu.is_gt, GT[:], sl(CN, j), NBEST[:])
                    for i in range(3):
                        tt(Alu.subtract, SC1[:], sl(M9, j * 3 + i), sl(V3, i))
                        tt(Alu.mult, SC1[:], SC1[:], GT[:])
                        tt(Alu.add, sl(V3, i), sl(V3, i), SC1[:])
                    tt(Alu.max, NBEST[:], NBEST[:], sl(CN, j))
                ts(SC1[:], NBEST[:], 1e-37, Alu.max)
                act(SC2[:], SC1[:], Act.Sqrt)
                nc.vector.reciprocal(SC2[:], SC2[:])
                for i in range(3):
                    tt(Alu.mult, sl(V3, i), sl(V3, i), SC2[:])

                # mode of semantic class (ties -> smallest)
                tt(Alu.subtract, BEST[:], NV[:], sl(OH, 0))
                for k in (1, 2, 3):
                    tt(Alu.subtract, BEST[:], BEST[:], sl(OH, k))
                nc.vector.memset(MODE[:], 0.0)
                for k in range(1, 5):
                    ck = sl(OH, k - 1)
                    tt(Alu.is_gt, GT[:], ck, BEST[:])
                    nc.vector.tensor_scalar(out=KT[:], in0=MODE[:],
                                            scalar1=-1.0, scalar2=float(k),
                                            op0=Alu.mult, op1=Alu.add)
                    tt(Alu.mult, KT[:], KT[:], GT[:])
                    tt(Alu.add, MODE[:], MODE[:], KT[:])
                    tt(Alu.max, BEST[:], BEST[:], ck)
                return DIRWT

            def pass_b(t):
                lb = int(Lb[t])
                xc = xcs[t]
                xcx = xc[:, 0:lb]; xcy = xc[:, lb:2 * lb]
                xcz = xc[:, 2 * lb:3 * lb]
                T = sp.tile([P, lb], dt.float32, tag="T", name=f"T{t}")
                S2 = sp.tile([P, lb], dt.float32, tag="S2", name=f"S2_{t}")
                S2b = sp.tile([P, lb], dt.float32, tag="S2b", name=f"S2b{t}")
                R = sp.tile([P, lb], dt.float32, tag="R", name=f"R{t}")
                sq3 = sq.tile([P, 3 * lb], dt.float32, tag="sq3", name=f"sq3{t}")
                nc.vector.tensor_scalar(out=T[:], in0=xcx,
                                        scalar1=V3[:, 0 * NT + t:0 * NT + t + 1],
                                        scalar2=None, op0=Alu.mult)
                stt(T[:], xcy, V3[:, 1 * NT + t:1 * NT + t + 1],
                    Alu.mult, Alu.add, T[:])
                stt(T[:], xcz, V3[:, 2 * NT + t:2 * NT + t + 1],
                    Alu.mult, Alu.add, T[:])
                # squares of the centered coords on the scalar engine
                for i, src in enumerate((xcx, xcy, xcz)):
                    act(sq3[:, i * lb:(i + 1) * lb], src, Act.Square)
                tt(Alu.add, S2[:], sq3[:, 0:lb], sq3[:, lb:2 * lb])
                tt(Alu.add, S2[:], S2[:], sq3[:, 2 * lb:3 * lb])
                stt(S2b[:], T[:], -1.0, Alu.mult, Alu.mult, T[:])
                tt(Alu.add, S2[:], S2[:], S2b[:])
                ts(S2[:], S2[:], 0.0, Alu.max)
                act(R[:], S2[:], Act.Sqrt)
                stt(S2b[:], T[:], 1.0, Alu.mult, Alu.mult, R[:],
                    accum=SCRAW[:, t:t + 1])

            def sign_phase(DIRWT):
                def tmp(tag, k=1):
                    return ret.tile([P, k * NT], dt.float32, tag=tag, name=tag)

                def sl(T, i):
                    return T[:, i * NT:(i + 1) * NT]

                T0 = tmp("T0"); CC = tmp("CC"); R0 = tmp("R0")
                SCV = tmp("SCV"); FAC = tmp("FAC"); SC9 = tmp("SC9")
                GT9 = tmp("GT9"); NPAD = tmp("NPAD")
                tt(Alu.mult, T0[:], sl(CEN, 0), sl(V3, 0))
                tt(Alu.mult, SC9[:], sl(CEN, 1), sl(V3, 1))
                tt(Alu.add, T0[:], T0[:], SC9[:])
                tt(Alu.mult, SC9[:], sl(CEN, 2), sl(V3, 2))
                tt(Alu.add, T0[:], T0[:], SC9[:])
                ts(T0[:], T0[:], -1.0, Alu.mult)
                tt(Alu.mult, CC[:], sl(CEN, 0), sl(CEN, 0))
                tt(Alu.mult, SC9[:], sl(CEN, 1), sl(CEN, 1))
                tt(Alu.add, CC[:], CC[:], SC9[:])
                tt(Alu.mult, SC9[:], sl(CEN, 2), sl(CEN, 2))
                tt(Alu.add, CC[:], CC[:], SC9[:])
                tt(Alu.mult, SC9[:], T0[:], T0[:])
                tt(Alu.subtract, R0[:], CC[:], SC9[:])
                ts(R0[:], R0[:], 0.0, Alu.max)
                act(R0[:], R0[:], Act.Sqrt)
                for t in range(NT):
                    nc.vector.tensor_scalar(
                        out=NPAD[:, t:t + 1],
                        in0=NV[:, t:t + 1], scalar1=-1.0,
                        scalar2=float(int(Lb[t])), op0=Alu.mult, op1=Alu.add)
                tt(Alu.mult, SC9[:], T0[:], R0[:])
                tt(Alu.mult, SC9[:], SC9[:], NPAD[:])
                tt(Alu.subtract, SCV[:], SCRAW[:], SC9[:])
                ts(GT9[:], SCV[:], 0.0, Alu.is_lt)
                nc.vector.tensor_scalar(out=GT9[:], in0=GT9[:], scalar1=-2.0,
                                        scalar2=1.0, op0=Alu.mult, op1=Alu.add)
                tt(Alu.mult, FAC[:], DIRWT[:], GT9[:])
                for i in range(3):
                    tt(Alu.mult, sl(V3, i), sl(V3, i), FAC[:])
                for j, pl in [(0, sl(CEN, 0)), (1, sl(CEN, 1)), (2, sl(CEN, 2)),
                              (3, sl(B6, 0)), (4, sl(B6, 1)), (5, sl(B6, 2)),
                              (6, sl(B6, 1)), (7, sl(B6, 3)), (8, sl(B6, 4)),
                              (9, sl(B6, 2)), (10, sl(B6, 4)), (11, sl(B6, 5)),
                              (12, sl(V3, 0)), (13, sl(V3, 1)), (14, sl(V3, 2)),
                              (15, NV[:]), (16, MEANV[:]), (17, STDV[:]),
                              (18, MODE[:])]:
                    nc.sync.dma_start(out=res[:, j * NT:(j + 1) * NT], in_=pl)

            for t in range(NT):
                load_and_pass_a(t)
            DIRWT = cluster_math()
            for t in range(NT):
                pass_b(t)
            sign_phase(DIRWT)

    nc.compile()
    return nc


_cache = {}
_last = None


def kernel(data, clust_idx, clust_len):
    global N, C, L, NT
    data = np.asarray(data)
    clust_idx = np.asarray(clust_idx)
    N = int(data.shape[0])
    C, L = int(clust_idx.shape[0]), int(clust_idx.shape[1])
    assert C % (P * N_CORES) == 0, f"cluster count {C} not divisible by {P * N_CORES}"
    NT = C // (P * N_CORES)
    gst, nvecs, Lb, S, ids = _host_prep(data, clust_idx, clust_len)

    key = tuple(int(x) for x in Lb)
    if key not in _cache:
        _cache[key] = _build_program(Lb, S)
    nc = _cache[key]

    from concourse.bass_utils import run_bass_kernel_spmd
    in_maps = [{"gst": gst[c], "nvec": nvecs[c]} for c in range(N_CORES)]
    global _last
    _last = (nc, in_maps)
    res = run_bass_kernel_spmd(nc, in_maps, list(range(N_CORES)))

    out = np.zeros((C, 19), dtype=f32)
    for core in range(N_CORES):
        r = res.results[core]["res"].reshape(P, 19, NT)
        for t in range(NT):
            out[ids[core, t]] = r[:, :, t]
    return out
